# revision 2
# baseline (speedup 1.0000x reference)
"""Masked multi-head attention on 8 Trainium2 NeuronCores (Bass/Tile).

Problem: Q,K,V [2, 16, 2048, 64] f32, mask [2, 1, 2048, 2048] bool ->
softmax(where(mask, -inf, QK^T) / sqrt(64)) @ V, computed as one SPMD Bass
program over 8 cores; each core owns 4 heads of one batch ((B,H) sharding).

Per-core kernel (per head, per 512-wide q-chunk "unit"):
  - scores^T[k, q] = K^T Q: fp32r matmuls with the D=64 contraction row-packed
    two k-blocks at a time into PE row groups (0,0)/(64,0), into PSUM score
    tiles covering 3 (or 2) k-blocks each.
  - exp on the Scalar engine straight off the raw scores (scale=1/8,
    PSUM -> SBUF, bf16 out). No mask beforehand and no row max: unmasked
    logits are O(6), so exp is safe, and softmax is shift-invariant.
  - mask applied POST-exp: one int16 tensor_sub on the bf16 bit pattern
    (0 keeps the weight, 8192 scales it by 2^-64 ~ 0). All operands are
    2-byte SBUF so the DVE runs in its 2x perf mode -- ~2.4x cheaper than
    the fp32 PSUM mask-add it replaces, and the Scalar engine (the real
    bottleneck: 16.8M exps/core on the only engine with exp) does no extra
    work.
  - O^T = V_aug^T @ P^T via bf16 matmuls accumulating over k in PSUM, where
    V_aug has a ones column appended: row 64 of the accumulator is the
    softmax denominator for free (consistent with the masked weights).
  - PE transposes [65 x 128] tiles back to [q, d], one DVE reciprocal and one
    broadcasted DVE multiply normalize, DMA out (p-major, unshuffled on host).
  - DMA order: head 0's Q/K/V first so compute starts immediately; the int16
    mask bias streams in afterwards in per-group chunks.
"""

import sys

sys.path.insert(0, "/opt/trn_rl_repo")

from contextlib import ExitStack

import numpy as np
import ml_dtypes

N_CORES = 8
B, HFULL, S, D = 2, 16, 2048, 64
H = (B * HFULL) // N_CORES  # heads per core
QC = 512
KB = S // 128
NQC = S // QC
NJ = QC // 128
GROUPS = (3, 3, 3, 3, 2, 2)  # k-blocks per PSUM score tile
MASK_SUB_I16 = 8192  # subtract from bf16 bits: multiplies the weight by 2^-64

# kept for test.py compatibility
MASK_ON_PE = 0.0
G = 3

_STATE = {}


def _build_program():
    import concourse.bass as bass
    import concourse.tile as tile
    from concourse import bacc, mybir
    from concourse.masks import make_identity

    F32 = mybir.dt.float32
    F32R = mybir.dt.float32r
    BF16 = mybir.dt.bfloat16
    I16 = mybir.dt.int16

    nc = bacc.Bacc(
        "TRN2", target_bir_lowering=False, debug=False, enable_partition_id=False
    )

    glist, kb0 = [], 0
    for n in GROUPS:
        glist.append(list(range(kb0, kb0 + n)))
        kb0 += n
    assert kb0 == KB
    max_g = max(len(g) for g in glist)

    qkt = nc.dram_tensor("qkt", [H, 64, 2, S], F32R, kind="ExternalInput").ap()
    vaug = nc.dram_tensor("vaug", [H, KB, 128, D + 1], BF16, kind="ExternalInput").ap()
    mbd = nc.dram_tensor("mbd", [128, KB, S], I16, kind="ExternalInput").ap()
    out = nc.dram_tensor("out", [H, NQC, 128, NJ, D], F32, kind="ExternalOutput").ap()

    with tile.TileContext(nc) as tc, ExitStack() as ctx:
        const_pool = ctx.enter_context(tc.tile_pool(name="const", bufs=1))
        mb_pool = ctx.enter_context(tc.tile_pool(name="mbpool", bufs=1))
        qk_pool = ctx.enter_context(tc.tile_pool(name="qkp", bufs=2))
        v_pool = ctx.enter_context(tc.tile_pool(name="vp", bufs=2))
        slab_pool = ctx.enter_context(tc.tile_pool(name="slab", bufs=2 * len(glist)))
        o_pool = ctx.enter_context(tc.tile_pool(name="op", bufs=2))
        small_pool = ctx.enter_context(tc.tile_pool(name="smallp", bufs=8))
        ps_score_pool = ctx.enter_context(
            tc.tile_pool(name="psscore", bufs=2, space="PSUM")
        )
        ps_small_pool = ctx.enter_context(
            tc.tile_pool(name="pssmall", bufs=2, space="PSUM")
        )

        ident_f = const_pool.tile([128, 128], F32)
        make_identity(nc, ident_f)

        # head 0's inputs first so compute starts immediately
        qk0 = qk_pool.tile([128, 2, S], F32R, tag="qk")
        nc.sync.dma_start(qk0[:64, :, :], qkt[0])
        nc.sync.dma_start(qk0[64:, :, :], qkt[0])
        v0 = v_pool.tile([128, KB, D + 1], BF16, tag="v")
        nc.gpsimd.dma_start(v0[:], vaug[0].rearrange("kb p d -> p kb d"))

        mbd_t = mb_pool.tile([128, KB, S], I16, tag="mbd")
        for g in glist:
            nc.sync.dma_start(mbd_t[:, g[0] : g[0] + len(g)], mbd[:, g[0] : g[0] + len(g)])

        for h in range(H):
            if h == 0:
                qk_t, v_t = qk0, v0
            else:
                qk_t = qk_pool.tile([128, 2, S], F32R, tag="qk")
                nc.sync.dma_start(qk_t[:64, :, :], qkt[h])
                nc.sync.dma_start(qk_t[64:, :, :], qkt[h])
                v_t = v_pool.tile([128, KB, D + 1], BF16, tag="v")
                nc.gpsimd.dma_start(v_t[:], vaug[h].rearrange("kb p d -> p kb d"))
            qt_t = qk_t[:, 0, :]
            kt_t = qk_t[:, 1, :]

            for qc in range(NQC):
                qsl = bass.ts(qc, QC)
                slabs = []
                for kbs in glist:
                    gg = len(kbs)
                    ps = ps_score_pool.tile([128, max_g, QC], F32, tag="ps")
                    for i, kb in enumerate(kbs):
                        half = kb % 2
                        lo, hi = half * 64, half * 64 + 64
                        nc.tensor.matmul(
                            ps[:, i, :],
                            kt_t[lo:hi, bass.ts(kb, 128)],
                            qt_t[lo:hi, qsl],
                            start=True,
                            stop=True,
                        )
                    slab = slab_pool.tile([128, max_g, QC], BF16, tag="slab")
                    nc.scalar.activation(
                        slab[:, :gg, :],
                        ps[:, :gg, :],
                        mybir.ActivationFunctionType.Exp,
                        scale=0.125,
                    )
                    si = slab[:, :gg, :].bitcast(I16)
                    nc.vector.tensor_sub(
                        si, si, mbd_t[:, kbs[0] : kbs[0] + gg, qsl]
                    )
                    slabs.append(slab)

                ps_o = ps_small_pool.tile([D + 1, QC], F32, tag="pssmall")
                kb_src = [
                    (gi, i) for gi, kbs in enumerate(glist) for i in range(len(kbs))
                ]
                for kb in range(KB):
                    gi, i = kb_src[kb]
                    nc.tensor.matmul(
                        ps_o[:],
                        v_t[:, kb, :],
                        slabs[gi][:, i, :],
                        start=(kb == 0),
                        stop=(kb == KB - 1),
                    )
                o_sb = o_pool.tile([D + 1, QC], F32, tag="osb")
                nc.vector.tensor_copy(o_sb[:], ps_o[:])

                out_sb = o_pool.tile([128, NJ, D], F32, tag="outsb")
                ps_t = ps_small_pool.tile([128, NJ, D + 1], F32, tag="pssmall")
                for j in range(NJ):
                    nc.tensor.transpose(
                        ps_t[:, j, :],
                        o_sb[:, bass.ts(j, 128)],
                        ident_f[: D + 1, : D + 1],
                    )
                rcp = small_pool.tile([128, NJ], F32, tag="rcp")
                nc.vector.reciprocal(rcp[:], ps_t[:, :, D])
                nc.vector.tensor_mul(
                    out_sb[:], ps_t[:, :, :D], rcp[:].broadcast_to((128, NJ, D))
                )
                nc.gpsimd.dma_start(out[h, qc], out_sb[:])

    nc.compile()
    return nc


class _Runner:
    """shard_map jit over the 8 NeuronCores, reusable across calls."""

    def __init__(self, nc):
        import jax
        from jax.sharding import Mesh, PartitionSpec
        from jax.experimental.shard_map import shard_map
        from concourse import mybir
        from concourse.bass2jax import _bass_exec_p, install_neuronx_cc_hook

        install_neuronx_cc_hook()
        self.jax = jax

        in_names, out_names, out_avals, zero_outs = [], [], [], []
        for alloc in nc.m.functions[0].allocations:
            if not isinstance(alloc, mybir.MemoryLocationSet):
                continue
            name = alloc.memorylocations[0].name
            if alloc.kind == "ExternalInput":
                in_names.append(name)
            elif alloc.kind == "ExternalOutput":
                shape = tuple(alloc.tensor_shape)
                dtype = mybir.dt.np(alloc.dtype)
                out_names.append(name)
                out_avals.append(jax.core.ShapedArray(shape, dtype))
                zero_outs.append(np.zeros(shape, dtype))
        self.in_names = in_names
        self.out_names = out_names
        self.out_avals = out_avals
        self.zero_outs = zero_outs
        all_in_names = in_names + out_names

        def _body(*args):
            outs = _bass_exec_p.bind(
                *args,
                out_avals=tuple(out_avals),
                in_names=tuple(all_in_names),
                out_names=tuple(out_names),
                lowering_input_output_aliases=(),
                sim_require_finite=True,
                sim_require_nnan=True,
                nc=nc,
            )
            return tuple(outs)

        devices = jax.devices()[:N_CORES]
        assert len(devices) == N_CORES, f"need {N_CORES} cores, saw {len(devices)}"
        mesh = Mesh(np.asarray(devices), ("core",))
        n_args = len(in_names) + len(out_names)
        self.sharded = jax.jit(
            shard_map(
                _body,
                mesh=mesh,
                in_specs=(PartitionSpec("core"),) * n_args,
                out_specs=(PartitionSpec("core"),) * len(out_names),
                check_rep=False,
            ),
            keep_unused=True,
        )

    def run(self, in_maps):
        jax = self.jax
        args = [
            np.concatenate([np.asarray(m[name]) for m in in_maps], axis=0)
            for name in self.in_names
        ]
        args += [
            np.zeros((N_CORES * z.shape[0], *z.shape[1:]), z.dtype)
            for z in self.zero_outs
        ]
        outs = self.sharded(*args)
        jax.block_until_ready(outs)
        return [
            {
                name: np.asarray(outs[i]).reshape(
                    N_CORES, *self.out_avals[i].shape
                )[c]
                for i, name in enumerate(self.out_names)
            }
            for c in range(N_CORES)
        ]


def _host_pack(Q, K, V, mask, core):
    hpc = H
    flat = core * hpc
    b = flat // HFULL
    h0 = flat % HFULL

    q = np.ascontiguousarray(Q[b, h0 : h0 + hpc])
    k = np.ascontiguousarray(K[b, h0 : h0 + hpc])
    v = np.ascontiguousarray(V[b, h0 : h0 + hpc])
    m = mask[b, 0]

    qkt = np.stack([q.transpose(0, 2, 1), k.transpose(0, 2, 1)], axis=2)

    vr = v.reshape(hpc, KB, 128, D)
    va = np.concatenate([vr, np.ones((hpc, KB, 128, 1), np.float32)], axis=-1).astype(
        ml_dtypes.bfloat16
    )

    mT = np.ascontiguousarray(m.T).reshape(KB, 128, S)
    md = np.where(mT, np.int16(MASK_SUB_I16), np.int16(0)).transpose(1, 0, 2)

    return {
        "qkt": np.ascontiguousarray(qkt),
        "vaug": np.ascontiguousarray(va),
        "mbd": np.ascontiguousarray(md),
    }


def _get_runner():
    if "runner" not in _STATE:
        _STATE["runner"] = _Runner(_build_program())
    return _STATE["runner"]


def kernel(Q, K, V, mask):
    Q = np.asarray(Q, dtype=np.float32)
    K = np.asarray(K, dtype=np.float32)
    V = np.asarray(V, dtype=np.float32)
    mask = np.asarray(mask).astype(bool)
    assert Q.shape == (B, HFULL, S, D), f"unexpected Q shape {Q.shape}"
    assert mask.shape == (B, 1, S, S), f"unexpected mask shape {mask.shape}"

    runner = _get_runner()
    in_maps = [_host_pack(Q, K, V, mask, c) for c in range(N_CORES)]
    results = runner.run(in_maps)

    out = np.empty((B, HFULL, S, D), np.float32)
    for core in range(N_CORES):
        flat = core * H
        b = flat // HFULL
        h0 = flat % HFULL
        # [H, NQC, 128, NJ, D] p-major -> [H, S, D]
        r = results[core]["out"].transpose(0, 1, 3, 2, 4).reshape(H, S, D)
        out[b, h0 : h0 + H] = r
    return out
